# revision 20
# baseline (speedup 1.0000x reference)
"""Transformer block Bass kernel for trn2 — fp8 DoubleRow edition.

Per batch element (tokens padded 523 -> 528):
  xc token-major bf16 (5 tiles [128,768])
  LN1 (bn_stats/bn_aggr; rstd = exp(-0.5*ln(var+eps)) keeps ACT on the
  exp table all phase) -> h_tm bf16 -> PE-transpose -> hfm fp8 [128,6,528]
  QKV via fp8 DoubleRow (K=768 as 3 pairs of 128-part chunks):
    q,k -> bf16 feature-major tiles (+ per-partition bias)
    v   -> vaug fp8 token-major pair tiles, head-slot stride 80, col 64 = 1.0
  attention: scoresT bf16 (K=64, no DR), exp -> expt fp8 pair tiles,
    o_ps = vaug.T @ expt via DR (2 pairs + 11-token tail), row 64 = denom,
    normalize (reciprocal + bcast + DVE mult) -> ofm fp8 pair tiles
  proj fp8 DR + residual (xc pre-biased with b_proj) -> x1 bf16
  LN2 -> hfm2 bf16, FC1 bf16 + Gelu -> gelu fp8 pair tiles,
  FC2 fp8 DR (K=3072 as 12 pairs) + residual (x1 pre-biased with b_fc2)
  Weights quantized fp8 e4m3 unscaled (sigma=0.02 straddles the denormal
  boundary; quant err ~3% either way). FC1 stays bf16 for accuracy.

Schedule per iteration (issue order): AV+normalize(b) | transpose+QKV(b+1)
(LN1(b+1) stats were hoisted to the end of iteration b-1 so the PE never
waits on DVE) | scores+exp(b+1) interleaved in pairs-of-heads with
proj/LN2/FC1/FC2(b) so the ACT-bound exp stream hides under FC matmuls.
The 11-token fc2 tails of all batches run as one deferred epilogue GEMM.
All psum chunks are >=264 cycles so every LDWEIGHTS stays hidden; scores
psums share the fc1 pool so the tile scheduler can't interleave Exp and
Gelu on ACT (table loads cost 1283ns).
"""

import numpy as np
import ml_dtypes
import concourse.bass as bass
import concourse.mybir as mybir
import concourse.tile as tile
from concourse import bacc
from concourse.masks import make_identity

F32 = mybir.dt.float32
BF16 = mybir.dt.bfloat16
FP8 = mybir.dt.float8e4
AF = mybir.ActivationFunctionType
OP = mybir.AluOpType
PM = mybir.MatmulPerfMode

C = 768
H = 12
Dh = 64
HID = 3072
N0 = 513
T = 10
NV = 523          # valid tokens
NT = 528          # padded tokens
EPS = 1e-5
SCALE = Dh ** -0.5
HS = 80           # vaug head-slot stride (65 used, 16B-aligned)

# token tiles (start, partitions); valid k-token counts per tile
TOKT = [(0, 128), (128, 128), (256, 128), (384, 128), (512, 16)]
TV = [128, 128, 128, 128, 11]
FQ264 = [(0, 264), (264, 264)]      # 1-bank psum chunks for qkv
FC768 = [(0, 512), (512, 256)]      # free chunks over 768


def build_nc(NB: int):
    nc = bacc.Bacc("TRN2", target_bir_lowering=False)

    xs = nc.dram_tensor("xs", [NB, N0, C], BF16, kind="ExternalInput")
    gf = nc.dram_tensor("gf", [NB, C], BF16, kind="ExternalInput")
    pe = nc.dram_tensor("pe", [T, C], BF16, kind="ExternalInput")
    tpos = nc.dram_tensor("tpos", [T, C], BF16, kind="ExternalInput")
    s1 = nc.dram_tensor("s1", [1, 1], F32, kind="ExternalInput")
    wqkvT = nc.dram_tensor("wqkvT", [C, 3 * C], FP8, kind="ExternalInput")
    qkb = nc.dram_tensor("qkb", [128, 12], F32, kind="ExternalInput")
    vb = nc.dram_tensor("vb", [1, C], F32, kind="ExternalInput")
    wprojT = nc.dram_tensor("wprojT", [C, C], FP8, kind="ExternalInput")
    bproj = nc.dram_tensor("bproj", [1, C], BF16, kind="ExternalInput")
    wfc1T = nc.dram_tensor("wfc1T", [C, HID], BF16, kind="ExternalInput")
    fc1b = nc.dram_tensor("fc1b", [128, 24], F32, kind="ExternalInput")
    wfc2T = nc.dram_tensor("wfc2T", [HID, C], FP8, kind="ExternalInput")
    bfc2 = nc.dram_tensor("bfc2", [1, C], BF16, kind="ExternalInput")
    out = nc.dram_tensor("out", [NB, NV, C], F32, kind="ExternalOutput")
    xth = nc.dram_tensor("xth", [NB, 11, C], BF16, kind="Internal")

    with tile.TileContext(nc) as tc:
        with (
            tc.tile_pool(name="consts", bufs=1) as consts,
            tc.tile_pool(name="p1", bufs=1) as p1,
            tc.tile_pool(name="p2", bufs=2) as p2,
            tc.tile_pool(name="p3", bufs=4) as p3,
            tc.tile_pool(name="psE", bufs=3, space="PSUM") as pspE,
            tc.tile_pool(name="psF", bufs=2, space="PSUM") as pspF,
        ):
            # ---- constants / resident tiles ----
            ident = consts.tile([128, 128], BF16, tag="ident")
            make_identity(nc, ident)
            eps_sb = consts.tile([128, 1], F32, tag="eps")
            nc.vector.memset(eps_sb, EPS)
            qkb_sb = consts.tile([128, 12], F32, tag="qkb")
            nc.sync.dma_start(qkb_sb[:], qkb[:])
            fc1b_sb = consts.tile([128, 24], F32, tag="fc1b")
            nc.sync.dma_start(fc1b_sb[:], fc1b[:])
            # per-feature biases broadcast to all 128 partitions
            vb_r = consts.tile([1, C], F32, tag="vb_r")
            nc.sync.dma_start(vb_r[:], vb[:])
            vb_bc = consts.tile([128, C], F32, tag="vb_bc")
            nc.gpsimd.partition_broadcast(vb_bc[:], vb_r[:])
            bproj_r = consts.tile([1, C], BF16, tag="bproj_r")
            nc.sync.dma_start(bproj_r[:], bproj[:])
            bproj_bc = consts.tile([128, C], BF16, tag="bproj_bc")
            nc.gpsimd.partition_broadcast(bproj_bc[:], bproj_r[:])
            bfc2_r = consts.tile([1, C], BF16, tag="bfc2_r")
            nc.sync.dma_start(bfc2_r[:], bfc2[:])
            bfc2_bc = consts.tile([128, C], BF16, tag="bfc2_bc")
            nc.gpsimd.partition_broadcast(bfc2_bc[:], bfc2_r[:])
            # prompt+position sum, scaler broadcast to 10 partitions
            pp_sb = consts.tile([T, C], BF16, tag="pp")
            tp_sb = p2.tile([T, C], BF16, tag="small768")
            nc.sync.dma_start(pp_sb[:], pe[:])
            nc.sync.dma_start(tp_sb[:], tpos[:])
            nc.vector.tensor_tensor(pp_sb[:], pp_sb[:], tp_sb[:], op=OP.add)
            s_1 = consts.tile([1, 1], F32, tag="s1")
            nc.sync.dma_start(s_1[:], s1[:])
            s_sb = consts.tile([T, 1], F32, tag="ssb")
            nc.gpsimd.partition_broadcast(s_sb[:], s_1[:])
            # ---- resident weights ----
            wqkv_dr = consts.tile([128, 6, 3 * C], FP8, tag="wqkv")
            nc.sync.dma_start(wqkv_dr[:],
                              wqkvT.rearrange("(j p) m -> p j m", p=128))
            wproj_dr = consts.tile([128, 6, C], FP8, tag="wproj")
            nc.sync.dma_start(wproj_dr[:],
                              wprojT.rearrange("(j p) m -> p j m", p=128))
            wfc2_dr = consts.tile([128, 24, C], FP8, tag="wfc2")
            nc.sync.dma_start(wfc2_dr[:],
                              wfc2T.rearrange("(j p) m -> p j m", p=128))
            wfc1 = []
            for kt in range(6):
                w = consts.tile([128, HID], BF16, tag=f"wfc1_{kt}")
                nc.sync.dma_start(w[:], wfc1T[kt * 128:(kt + 1) * 128, :])
                wfc1.append(w)
            # deferred 11-token fc2 tails: batch slots at 32b (32-aligned)
            gtails = []
            for kp in range(12):
                g = consts.tile([128, 2, 128], FP8, tag=f"gt{kp}")
                nc.vector.memset(g[:], 0.0)
                gtails.append(g)


            def ln_phase(src_tiles, htag):
                """LayerNorm token-major -> bf16 h tiles (gamma folded on host).

                ACT Sqrt + DVE reciprocal for rstd (table swaps measured
                no worse than ln/exp tricks; sqrts stay contiguous)."""
                mvall = p2.tile([128, 5, 2], F32, tag="bnmv")
                nc.vector.memset(mvall[:], 1.0)
                for tt, (t0, TP) in enumerate(TOKT):
                    PV = TV[tt]
                    stats = p2.tile([128, 2, 6], F32, tag="bnst")
                    x3 = src_tiles[tt][:PV].rearrange("p (a b) -> p a b", b=384)
                    nc.vector.bn_stats(stats[:PV, 0, :], x3[:, 0, :])
                    nc.vector.bn_stats(stats[:PV, 1, :], x3[:, 1, :])
                    nc.vector.bn_aggr(mvall[:PV, tt, :], stats[:PV])
                # one batched sqrt for all 5 tiles keeps ACT table churn low
                nc.scalar.activation(mvall[:, :, 1], mvall[:, :, 1], AF.Sqrt,
                                     bias=eps_sb[:])
                nc.vector.reciprocal(mvall[:, :, 1], mvall[:, :, 1])
                h_tiles = []
                for tt, (t0, TP) in enumerate(TOKT):
                    PV = TV[tt]
                    h = p1.tile([128, C], BF16, tag=f"{htag}{tt}", name=f"h{tt}")
                    if TP > PV:
                        nc.vector.memset(h[:TP, :], 0.0)
                    nc.vector.tensor_scalar(
                        out=h[:PV], in0=src_tiles[tt][:PV],
                        scalar1=mvall[:PV, tt, 0:1], scalar2=mvall[:PV, tt, 1:2],
                        op0=OP.subtract, op1=OP.mult)
                    h_tiles.append(h)
                return h_tiles

            def phase0(b):
                xc = [p3.tile([128, C], BF16, tag=f"resid{t}", name=f"xc{t}")
                      for t in range(5)]
                nc.sync.dma_start(xc[0][0:1, :], xs[b, 0:1, :])
                nc.sync.dma_start(xc[0][11:128, :], xs[b, 1:118, :])
                nc.sync.dma_start(xc[1][:], xs[b, 118:246, :])
                nc.sync.dma_start(xc[2][:], xs[b, 246:374, :])
                nc.sync.dma_start(xc[3][:], xs[b, 374:502, :])
                nc.vector.memset(xc[4][0:16, :], 0.0)
                nc.sync.dma_start(xc[4][0:11, :], xs[b, 502:513, :])
                gf1 = p2.tile([1, C], BF16, tag="small768")
                nc.sync.dma_start(gf1[:], gf[b:b + 1, :])
                gfb = p2.tile([T, C], BF16, tag="small768")
                nc.gpsimd.partition_broadcast(gfb[:], gf1[:])
                nc.vector.tensor_scalar_mul(gfb[:], gfb[:], s_sb[:])
                nc.vector.tensor_tensor(gfb[:], gfb[:], pp_sb[:], op=OP.add)
                nc.sync.dma_start(xc[0][1:11, :], gfb[:])
                return xc

            def s1_pre(b):
                """phase0 + LN1 (DVE-only) — hoisted an iteration early so
                the PE never waits on LN stats at iteration start."""
                xc = phase0(b)
                h1 = ln_phase(xc, "htm")
                # fold b_proj into the residual now that LN1 has consumed xc
                for tt, (t0, TP) in enumerate(TOKT):
                    nc.vector.tensor_tensor(xc[tt][:TP], xc[tt][:TP],
                                            bproj_bc[:TP], op=OP.add)
                return xc, h1

            def s1_main(xc, h1):
                """transpose + QKV (PE work) for a batch prepped by s1_pre."""
                hfm = p1.tile([128, 6, NT], FP8, tag="hfm8", name="hfm8")
                for ct in range(6):
                    tp_ps = pspF.tile([128, 528], BF16, tag="early")
                    for tt, (t0, TP) in enumerate(TOKT):
                        nc.tensor.transpose(
                            tp_ps[:, t0:t0 + TP],
                            h1[tt][:TP, ct * 128:(ct + 1) * 128],
                            ident[:TP, :TP])
                    nc.vector.tensor_copy(hfm[:, ct, :], tp_ps[:])
                qk = []
                for m in range(12):
                    q = p1.tile([128, NT], FP8, tag=f"qk{m}", name=f"qk{m}")
                    for (f0, fl) in FQ264:
                        ps = pspF.tile([128, 264], F32, tag="early")
                        for j in range(3):
                            nc.tensor.matmul(
                                ps[:, :fl],
                                wqkv_dr[:, 2 * j:2 * j + 2,
                                        m * 128:(m + 1) * 128],
                                hfm[:, 2 * j:2 * j + 2, f0:f0 + fl],
                                start=(j == 0), stop=(j == 2),
                                perf_mode=PM.DoubleRow)
                        nc.scalar.activation(q[:, f0:f0 + fl], ps[:, :fl],
                                             AF.Identity,
                                             bias=qkb_sb[:, m:m + 1])
                    qk.append(q)
                # v: token-major fp8 pair tiles, head slots of stride HS
                vp = [p1.tile([128, 2, H * HS], FP8, tag=f"vaug{j}",
                              name=f"vp{j}") for j in range(2)]
                vtail = p1.tile([16, H * HS], FP8, tag="vaug2", name="vtail")
                for j in range(2):
                    v4 = vp[j].rearrange("p i (h e) -> p i h e", e=HS)
                    nc.vector.memset(v4[:, :, :, 64:65], 1.0)
                vt4 = vtail.rearrange("p (h e) -> p h e", e=HS)
                nc.vector.memset(vt4[:11, :, 64:65], 1.0)
                for tt, (t0, TP) in enumerate(TOKT):
                    PV = TV[tt]
                    ps = pspE.tile([128, C], F32, tag="ps")
                    for (f0, fl) in FC768:
                        for j in range(3):
                            nc.tensor.matmul(
                                ps[:TP, f0:f0 + fl],
                                hfm[:, 2 * j:2 * j + 2, t0:t0 + TP],
                                wqkv_dr[:, 2 * j:2 * j + 2,
                                        2 * C + f0:2 * C + f0 + fl],
                                start=(j == 0), stop=(j == 2),
                                perf_mode=PM.DoubleRow)
                    s3 = ps[:PV].rearrange("p (h e) -> p h e", e=64)
                    b3 = vb_bc[:PV].rearrange("p (h e) -> p h e", e=64)
                    if tt < 4:
                        dst = vp[tt // 2][:, tt % 2, :].rearrange(
                            "p (h e) -> p h e", e=HS)[:PV, :, 0:64]
                    else:
                        dst = vt4[:PV, :, 0:64]
                    nc.vector.tensor_tensor(dst, s3, b3, op=OP.add)
                return xc, qk, (vp, vtail)

            def alloc_exps():
                exps = []
                for h in range(H):
                    ep = [p1.tile([128, 2, NT], FP8, tag=f"ex{h}_{j}",
                                  name=f"ex{h}_{j}") for j in range(2)]
                    etail = p1.tile([16, NT], FP8, tag=f"ex{h}t",
                                    name=f"ex{h}t")
                    exps.append((ep, etail))
                return exps

            def scores_chunk(qk, exps, heads):
                """scoresT + exp for a pair of heads (interleaved with mlp)."""
                for h in heads:
                    t, po = h // 2, (h % 2) * 64
                    ep, etail = exps[h]
                    for kt, (k0, TP) in enumerate(TOKT):
                        KV = TV[kt]
                        sc = pspE.tile([128, 2, 512], F32, tag="ps", name="sc")
                        for i in range(2):
                            nc.tensor.matmul(
                                sc[:KV, i, 0:264],
                                qk[6 + t][po:po + 64, k0:k0 + KV],
                                qk[t][po:po + 64, i * 264:(i + 1) * 264],
                                start=True, stop=True)
                        dst = (ep[kt // 2][:KV, kt % 2, :] if kt < 4
                               else etail[:KV, :])
                        nc.scalar.activation(
                            dst.rearrange("p (a b) -> p a b", b=264),
                            sc[:KV, :, 0:264], AF.Exp, scale=SCALE)

            def attn_av(vaug, exps):
                """AV via fp8 DR + softmax normalize -> ofm fp8 pair tiles."""
                vp, vtail = vaug
                op = [p1.tile([128, 2, NT], FP8, tag=f"ofm{j}", name=f"ofm{j}")
                      for j in range(3)]
                for h in range(H):
                    po = (h % 2) * 64
                    ep, etail = exps[h]
                    o_ps = pspE.tile([128, 2, 512], F32, tag="ps")
                    for i in range(2):
                        for j in range(2):
                            nc.tensor.matmul(
                                o_ps[:65, i, 0:264],
                                vp[j][:, :, h * HS:h * HS + 65],
                                ep[j][:, :, i * 264:(i + 1) * 264],
                                start=(j == 0), stop=False,
                                perf_mode=PM.DoubleRow)
                        nc.tensor.matmul(
                            o_ps[:65, i, 0:264],
                            vtail[0:11, h * HS:h * HS + 65],
                            etail[0:11, i * 264:(i + 1) * 264],
                            start=False, stop=True)
                    recip = p2.tile([1, NT], BF16, tag="recip")
                    with nc.allow_low_precision(reason="softmax recip to bf16"):
                        nc.vector.reciprocal(recip[:], o_ps[64:65, :, 0:264])
                    rb = p2.tile([64, NT], BF16, tag="rb")
                    nc.gpsimd.partition_broadcast(rb[:], recip[:])
                    nc.vector.tensor_tensor(
                        op[h // 4][po:po + 64, (h // 2) % 2, :],
                        o_ps[0:64, :, 0:264], rb[:], op=OP.mult)
                return op

            def proj_part(xcb, ofm):
                x1 = [p3.tile([128, C], BF16, tag=f"resid{t}", name=f"x1_{t}")
                      for t in range(5)]
                for tt, (t0, TP) in enumerate(TOKT):
                    ps = pspE.tile([128, C], F32, tag="ps")
                    for (f0, fl) in FC768:
                        for j in range(3):
                            nc.tensor.matmul(
                                ps[:TP, f0:f0 + fl],
                                ofm[j][:, :, t0:t0 + TP],
                                wproj_dr[:, 2 * j:2 * j + 2, f0:f0 + fl],
                                start=(j == 0), stop=(j == 2),
                                perf_mode=PM.DoubleRow)
                    nc.vector.tensor_tensor(x1[tt][:TP], ps[:TP], xcb[tt][:TP],
                                            op=OP.add)
                return x1

            def ln2_part(x1):
                h2 = ln_phase(x1, "htm")
                hfm2 = []
                for ct in range(6):
                    tp_ps = pspF.tile([128, 528], BF16, tag="early",
                                      name=f"tp{ct}")
                    for tt, (t0, TP) in enumerate(TOKT):
                        nc.tensor.transpose(
                            tp_ps[:, t0:t0 + TP],
                            h2[tt][:TP, ct * 128:(ct + 1) * 128],
                            ident[:TP, :TP])
                    hf = p1.tile([128, NT], BF16, tag=f"hfm2_{ct}",
                                 name=f"hfm{ct}")
                    nc.vector.tensor_copy(hf[:], tp_ps[:])
                    hfm2.append(hf)
                # fold b_fc2 into the residual now that LN2 has consumed x1
                for tt, (t0, TP) in enumerate(TOKT):
                    nc.vector.tensor_tensor(x1[tt][:TP], x1[tt][:TP],
                                            bfc2_bc[:TP], op=OP.add)
                return hfm2

            def fc1_part(hfm2, gp, ms):
                for m in ms:
                    ps = pspE.tile([128, 2, 512], F32, tag="ps")
                    for i in range(2):
                        for kt in range(6):
                            nc.tensor.matmul(
                                ps[:, i, 0:264],
                                wfc1[kt][:, m * 128:(m + 1) * 128],
                                hfm2[kt][:, i * 264:(i + 1) * 264],
                                start=(kt == 0), stop=(kt == 5))
                    nc.scalar.activation(
                        gp[m // 2][:, m % 2, :].rearrange("p (a b) -> p a b",
                                                          b=264),
                        ps[:, :, 0:264], AF.Gelu, bias=fc1b_sb[:, m:m + 1])

            def fc2_part(b, x1b, gp, tts):
                for tt in tts:
                    t0, TP = TOKT[tt]
                    ps = pspE.tile([128, C], F32, tag="ps")
                    for (f0, fl) in FC768:
                        for kp in range(12):
                            nc.tensor.matmul(
                                ps[:TP, f0:f0 + fl],
                                gp[kp][:, :, t0:t0 + TP],
                                wfc2_dr[:, 2 * kp:2 * kp + 2, f0:f0 + fl],
                                start=(kp == 0), stop=(kp == 11),
                                perf_mode=PM.DoubleRow)
                    ob = p2.tile([128, C], F32, tag="outsb")
                    nc.vector.tensor_tensor(ob[:TP], ps[:TP], x1b[tt][:TP],
                                            op=OP.add)
                    rows = min(TP, NV - t0)
                    nc.sync.dma_start(out[b, t0:t0 + rows, :], ob[:rows])

            # ---- schedule ----
            pres = {}
            pres[0] = s1_pre(0)
            cur = s1_main(*pres.pop(0))
            pres[1] = s1_pre(1)
            cur_exp = alloc_exps()
            scores_chunk(cur[1], cur_exp, range(H))
            for b in range(NB):
                xcb, qk, vaug = cur
                exps = cur_exp
                ofm = attn_av(vaug, exps)
                nxt = nxt_exp = None
                if b + 1 < NB:
                    nxt = s1_main(*pres.pop(b + 1))
                    nxt_exp = alloc_exps()

                    def sch(hs, _qk=nxt[1], _ex=nxt_exp):
                        scores_chunk(_qk, _ex, hs)
                else:
                    def sch(hs):
                        pass
                sch([0, 1])
                x1 = proj_part(xcb, ofm)
                sch([2, 3])
                hfm2 = ln2_part(x1)
                gp = [p1.tile([128, 2, NT], FP8, tag=f"gelu{j}", name=f"g{j}")
                      for j in range(12)]
                fc1_part(hfm2, gp, range(0, 24))
                nc.sync.dma_start(xth[b], x1[4][:11, :])
                for kp in range(12):
                    nc.vector.tensor_copy(gtails[kp][:, :, 32 * b:32 * b + 16],
                                          gp[kp][:, :, 512:528])
                sch([4, 5])
                fc2_part(b, x1, gp, [0])
                sch([6, 7])
                fc2_part(b, x1, gp, [1])
                sch([8, 9])
                fc2_part(b, x1, gp, [2])
                sch([10, 11])
                fc2_part(b, x1, gp, [3])
                if b + 2 < NB:
                    pres[b + 2] = s1_pre(b + 2)
                cur, cur_exp = nxt, nxt_exp

            # epilogue: one fc2 pass over all batches' 11-token tails
            ps = pspE.tile([128, C], F32, tag="ps", name="tailps")
            for (f0, fl) in FC768:
                for kp in range(12):
                    nc.tensor.matmul(
                        ps[:, f0:f0 + fl],
                        gtails[kp][:, :, :],
                        wfc2_dr[:, 2 * kp:2 * kp + 2, f0:f0 + fl],
                        start=(kp == 0), stop=(kp == 11),
                        perf_mode=PM.DoubleRow)
            for b in range(NB):
                tx = p2.tile([11, C], BF16, tag="small768", name="tx")
                nc.sync.dma_start(tx[:], xth[b])
                obt = p2.tile([16, C], F32, tag="outsb", name="obt")
                nc.vector.tensor_tensor(obt[:11], ps[32 * b:32 * b + 11, :],
                                        tx[:], op=OP.add)
                nc.sync.dma_start(out[b, 512:523, :], obt[:11])

    nc.compile()
    return nc


def prep_weights(inp):
    """Host-side: fold gammas into weights, pre-transpose, cast fp8/bf16."""
    f32 = np.float32
    g1 = np.asarray(inp["g1"], f32)
    b1 = np.asarray(inp["b1"], f32)
    g2 = np.asarray(inp["g2"], f32)
    b2 = np.asarray(inp["b2"], f32)
    w_qkv = np.asarray(inp["w_qkv"], f32)
    w_proj = np.asarray(inp["w_proj"], f32)
    w_fc1 = np.asarray(inp["w_fc1"], f32)
    w_fc2 = np.asarray(inp["w_fc2"], f32)
    bf = ml_dtypes.bfloat16
    f8 = ml_dtypes.float8_e4m3

    wqkv_g = w_qkv * g1[None, :]
    qkv_bias = w_qkv @ b1                       # [2304]
    wfc1_g = w_fc1 * g2[None, :]
    fc1_bias = np.asarray(inp["b_fc1"], f32) + w_fc1 @ b2

    d = {}
    d["wqkvT"] = np.ascontiguousarray(wqkv_g.T).astype(f8)
    d["qkb"] = np.ascontiguousarray(qkv_bias[:1536].reshape(12, 128).T)
    d["vb"] = qkv_bias[1536:].reshape(1, C).copy()
    d["wprojT"] = np.ascontiguousarray(w_proj.T).astype(f8)
    d["bproj"] = np.asarray(inp["b_proj"], f32).reshape(1, C).astype(bf)
    d["wfc1T"] = np.ascontiguousarray(wfc1_g.T).astype(bf)
    d["fc1b"] = np.ascontiguousarray(fc1_bias.reshape(24, 128).T)
    d["wfc2T"] = np.ascontiguousarray(w_fc2.T).astype(f8)
    d["bfc2"] = np.asarray(inp["b_fc2"], f32).reshape(1, C).astype(bf)
    d["pe"] = np.asarray(inp["prompt_emb"], f32).astype(bf)
    d["tpos"] = np.asarray(inp["token_position"], f32).astype(bf)
    d["s1"] = np.asarray(inp["scaler1"], f32).reshape(1, 1)
    return d


def make_in_maps(inp, n_cores=8):
    """Split batch across cores; returns list of per-core input dicts."""
    x = np.asarray(inp["x"], np.float32)
    gfeat = np.asarray(inp["global_feature"], np.float32)
    B = x.shape[0]
    nb = B // n_cores
    shared = prep_weights(inp)
    maps = []
    for c in range(n_cores):
        m = dict(shared)
        m["xs"] = np.ascontiguousarray(x[c * nb:(c + 1) * nb]).astype(ml_dtypes.bfloat16)
        m["gf"] = np.ascontiguousarray(gfeat[c * nb:(c + 1) * nb, 0, :]).astype(ml_dtypes.bfloat16)
        maps.append(m)
    return maps


_CACHED = {}


def _get_nc():
    if "nc" not in _CACHED:
        _CACHED["nc"] = build_nc(4)
    return _CACHED["nc"]


def kernel(**inputs):
    """Full-input transformer block on 8 NeuronCores (batch-parallel, 4/core)."""
    from concourse.bass_utils import run_bass_kernel_spmd

    n_cores = 8
    nc = _get_nc()
    maps = make_in_maps(inputs, n_cores=n_cores)
    res = run_bass_kernel_spmd(nc, maps, core_ids=list(range(n_cores)))
    out = np.concatenate([res.results[c]["out"] for c in range(n_cores)], axis=0)
    return out.astype(np.float32)
